# revision 1
# baseline (speedup 1.0000x reference)
"""Trainium2 Bass kernel for the differentiable gaussian-splat renderer.

Full-input contract: kernel(**inputs) takes the unsharded inputs and returns
the full [2*16, 3, 32, 32] output.

Math (per pose):
    cam = positions @ R.T + t ;  pj = (fx*cam_x/cam_z + cx, fy*cam_y/cam_z + cy)
    w[n, p] = op_n * exp(-0.5*((px-ax_n)^2 + (py-ay_n)^2)/s_n^2)
    img = (w.T @ colors) / (w.T @ 1 + 1e-8)

The gaussian weight is separable: w = op * wx[n,px] * wy[n,py], so instead of
N*HW exponentials we need N*(W + H) and the pixel accumulation becomes a
K=128-chunked matmul  out[py, (c,px)] += wy_chunk.T @ (ca_chunk (*) wx_chunk).

Sharding: 8 independent cores = 2 poses x 4 px-column blocks (32 px each).
No collectives; each core computes all 4096 gaussians for its (pose, px-block)
and writes a [128, 96] slab = (py, 32c+px_local). Host reassembles.

The exp argument g*(q'-ax')^2 (q' centered at 64) is evaluated as a matmul of
per-gaussian quadratic coefficients [g, -2*g*ax', g*ax'^2] against a
block-diagonal pixel basis [q'^2, q', 1]. For PE speed the coefficients are
split into 3 bf16 pieces each (exact to ~24 bits; pixel basis rows are
bf16-exact integers, q'^2 split into hi+lo rows), so the arg matmuls run at
1 cycle/column instead of fp32's 4. The main accumulation matmul runs on
fp32r (positive, well-conditioned sums).
"""

import numpy as np

H = 128
W = 128
FX = 120.0
FY = 120.0
CX = 64.0
CY = 64.0
N = 4096
NCHUNK = 32          # 4096 / 128
NPOSE = 2
PXB = 32             # px columns per core
NBLK = 4             # px blocks
F32 = np.float32

# main-matmul operand dtype: "fp32r" or "fp32"
MAIN_DTYPE = "fp32"

_CACHE = {}


def _quat2mat(q):
    q = np.asarray(q, dtype=np.float64)
    q = q / np.linalg.norm(q)
    w, x, y, z = q
    return np.array([
        [1 - 2 * (y * y + z * z), 2 * (x * y - z * w), 2 * (x * z + y * w)],
        [2 * (x * y + z * w), 1 - 2 * (x * x + z * z), 2 * (y * z - x * w)],
        [2 * (x * z - y * w), 2 * (y * z + x * w), 1 - 2 * (x * x + y * y)],
    ])


def _build_program():
    """Build the SPMD Bass/Tile program (same program on every core)."""
    import concourse.bacc as bacc
    import concourse.tile as tile
    import concourse.mybir as mybir
    from contextlib import ExitStack

    dt = mybir.dt.float32
    bf = mybir.dt.bfloat16
    dtm = mybir.dt.float32r if MAIN_DTYPE == "fp32r" else mybir.dt.float32
    nc = bacc.Bacc()

    # ---- DRAM I/O (per-core shapes) ----
    # inp128 cols: 0:128 pos4 | 128:256 colors1 | 256:288 opac |
    #              288:416 ident | 416:428 rb (pre-broadcast)
    inp128_d = nc.dram_tensor("inp128", [128, 428], dt, kind="ExternalInput").ap()
    # bas (bf16): rows 0:64 cols 0:512 = basis_y ; cols 512:768 = basis_x ;
    #             cols 768:896 = bf16 identity (for PE transposes)
    bas_d = nc.dram_tensor("bas", [128, 896], bf, kind="ExternalInput").ap()
    out_d = nc.dram_tensor("out", [128, 96], dt, kind="ExternalOutput").ap()

    mult = mybir.AluOpType.mult
    add = mybir.AluOpType.add
    sub = mybir.AluOpType.subtract
    EXP = mybir.ActivationFunctionType.Exp

    with tile.TileContext(nc) as tc, ExitStack() as ctx:
        const = ctx.enter_context(tc.tile_pool(name="const", bufs=1))
        work = ctx.enter_context(tc.tile_pool(name="work", bufs=1))
        xpool = ctx.enter_context(tc.tile_pool(name="xpool", bufs=8))
        psum_arg = ctx.enter_context(tc.tile_pool(name="psum_arg", bufs=2, space="PSUM"))
        psum_tp = ctx.enter_context(tc.tile_pool(name="psum_tp", bufs=2, space="PSUM"))
        psum_out = ctx.enter_context(tc.tile_pool(name="psum_out", bufs=1, space="PSUM"))

        po = psum_out.tile([128, 128], dt, tag="po")  # claim psum bank 0 first
        inp128 = const.tile([128, 428], dt, tag="inp128")
        nc.sync.dma_start(out=inp128[:], in_=inp128_d)
        bas = const.tile([128, 896], bf, tag="bas")
        nc.sync.dma_start(out=bas[:], in_=bas_d)
        ident_bf = bas[:, 768:896]

        colors1 = inp128[:, 128:256]
        opac = inp128[:, 256:288]
        ident = inp128[:, 288:416]
        rb = inp128[:, 416:428]
        # basis_y duplicated at rows 0:64 and 64:128 (matmul requires lhsT and
        # rhs to share a base partition; odd groups' lhsT sits at rows 64:128)
        basis_y2 = [bas[0:64, 0:512], bas[64:128, 0:512]]
        basis_x = bas[:, 512:768]

        xg = inp128[:, 0:32]
        yg = inp128[:, 32:64]
        zg = inp128[:, 64:96]
        sg = inp128[:, 96:128]

        # ---- projection: u,v,zc = A @ [x,y,z,1]; u-chain on DVE, v-chain on
        #      GpSimd so the serial chains run concurrently ----
        def lin3(eng, c0):
            acc = work.tile([128, 32], dt, tag=f"acc{c0}")
            eng.tensor_scalar(out=acc[:], in0=xg, scalar1=rb[:, c0:c0 + 1],
                              scalar2=rb[:, c0 + 3:c0 + 4], op0=mult, op1=add)
            t1 = work.tile([128, 32], dt, tag=f"t1{c0}")
            eng.tensor_scalar(out=t1[:], in0=yg, scalar1=rb[:, c0 + 1:c0 + 2],
                              scalar2=None, op0=mult)
            eng.tensor_add(out=acc[:], in0=acc[:], in1=t1[:])
            t2 = work.tile([128, 32], dt, tag=f"t2{c0}")
            eng.tensor_scalar(out=t2[:], in0=zg, scalar1=rb[:, c0 + 2:c0 + 3],
                              scalar2=None, op0=mult)
            eng.tensor_add(out=acc[:], in0=acc[:], in1=t2[:])
            return acc

        u = lin3(nc.vector, 0)
        v = lin3(nc.gpsimd, 4)
        zc = lin3(nc.vector, 8)
        zr = work.tile([128, 32], dt, tag="zr")
        nc.vector.reciprocal(out=zr[:], in_=zc[:])
        ax = work.tile([128, 32], dt, tag="ax")
        nc.vector.tensor_mul(out=ax[:], in0=u[:], in1=zr[:])
        ay = work.tile([128, 32], dt, tag="ay")
        nc.gpsimd.tensor_mul(out=ay[:], in0=v[:], in1=zr[:])

        s2 = work.tile([128, 32], dt, tag="s2")
        nc.gpsimd.tensor_mul(out=s2[:], in0=sg, in1=sg)
        gr = work.tile([128, 32], dt, tag="gr")
        nc.vector.reciprocal(out=gr[:], in_=s2[:])
        g = work.tile([128, 32], dt, tag="g")
        nc.vector.tensor_scalar(out=g[:], in0=gr[:], scalar1=-0.5, scalar2=None,
                                op0=mult)

        # ---- bf16 coef packs [128, 512], col 16*j + r; rows per chunk:
        #      (A1,A1,A2,A2,A3,A3,B1,B2,B3,C1,C2,C3,0,0,0,0) ----
        pack_x = const.tile([128, 512], bf, tag="packx")
        pack_y = const.tile([128, 512], bf, tag="packy")
        nc.gpsimd.memset(pack_x[:], 0.0)
        nc.gpsimd.memset(pack_y[:], 0.0)

        def prow(pk, r):
            # strided view: row r of each chunk -> [128, 32]
            return pk[:].rearrange("p (j r) -> p r j", r=16)[:, r, :]

        def split3(eng, src, pk, r0, name):
            """Write bf16 pieces of src to pack rows r0, r0+1, r0+2."""
            eng.tensor_copy(out=prow(pk, r0), in_=src[:])
            res1 = work.tile([128, 32], dt, tag=f"res1{name}")
            eng.tensor_tensor(out=res1[:], in0=src[:], in1=prow(pk, r0), op=sub)
            eng.tensor_copy(out=prow(pk, r0 + 1), in_=res1[:])
            res2 = work.tile([128, 32], dt, tag=f"res2{name}")
            eng.tensor_tensor(out=res2[:], in0=res1[:], in1=prow(pk, r0 + 1), op=sub)
            eng.tensor_copy(out=prow(pk, r0 + 2), in_=res2[:])

        # A pieces land in rows 0,1,2; remap to (A1,A1,A2,A2,A3,A3) and copy
        # the block to pack_y (A = g is shared between the axes).
        split3(nc.vector, g, pack_x, 0, "A")

        pxr_x = pack_x[:].rearrange("p (j r) -> p r j", r=16)
        pxr_y = pack_y[:].rearrange("p (j r) -> p r j", r=16)
        # duplicate A rows: rows written by split3 were 0,1,2 — remap:
        # shift row1->row2... simpler: rewrite A pieces at correct rows now.
        # (split3 wrote rows 0,1,2; we need them at 0,2,4 with dups at 1,3,5)
        nc.gpsimd.tensor_copy(out=pxr_x[:, 4, :], in_=pxr_x[:, 2, :])
        nc.gpsimd.tensor_copy(out=pxr_x[:, 2, :], in_=pxr_x[:, 1, :])
        nc.gpsimd.tensor_copy(out=pxr_x[:, 1, :], in_=pxr_x[:, 0, :])
        nc.gpsimd.tensor_copy(out=pxr_x[:, 3, :], in_=pxr_x[:, 2, :])
        nc.gpsimd.tensor_copy(out=pxr_x[:, 5, :], in_=pxr_x[:, 4, :])
        # copy A block (rows 0..5) to pack_y
        nc.gpsimd.tensor_copy(out=pxr_y[:, 0:6, :], in_=pxr_x[:, 0:6, :])

        def bc_coefs(eng, axy, pk, name):
            ga = work.tile([128, 32], dt, tag=f"ga{name}")
            eng.tensor_mul(out=ga[:], in0=g[:], in1=axy[:])
            B = work.tile([128, 32], dt, tag=f"B{name}")
            eng.tensor_scalar(out=B[:], in0=ga[:], scalar1=-2.0, scalar2=None,
                              op0=mult)
            C = work.tile([128, 32], dt, tag=f"C{name}")
            eng.tensor_mul(out=C[:], in0=ga[:], in1=axy[:])
            split3(eng, B, pk, 6, f"B{name}")
            split3(eng, C, pk, 9, f"C{name}")

        bc_coefs(nc.vector, ax, pack_x, "x")
        bc_coefs(nc.gpsimd, ay, pack_y, "y")

        # ---- transpose packs: 4 x [128,128] per axis -> coefT bf16 tiles ----
        def transpose_pack(pk, name):
            cts = []
            for t in range(4):
                tp = psum_tp.tile([128, 128], bf, tag="tp")
                nc.tensor.transpose(tp[:], pk[:, 128 * t:128 * t + 128], ident_bf)
                ct = const.tile([128, 128], bf, tag=f"coefT{name}{t}")
                nc.vector.tensor_copy(out=ct[:], in_=tp[:])
                cts.append(ct)
            return cts

        coefT_x = transpose_pack(pack_x, "x")
        coefT_y = transpose_pack(pack_y, "y")

        # ---- ca = colors1 * opac  [128, 128] (cols 32c+j) ----
        ca = const.tile([128, 128], dt, tag="ca")
        ca_r = ca[:].rearrange("p (c j) -> p c j", c=4)
        col_r = colors1.rearrange("p (c j) -> p c j", c=4)
        op_b = opac.unsqueeze(1).broadcast_to([128, 4, 32])
        nc.vector.tensor_mul(out=ca_r, in0=col_r, in1=op_b)

        # ---- wx args: 4 matmuls [128,128]x[128,256] -> one exp ----
        pa = psum_arg.tile([128, 1024], dt, tag="pa")
        for t in range(4):
            nc.tensor.matmul(pa[:, 256 * t:256 * t + 256], lhsT=coefT_x[t][:],
                             rhs=basis_x, start=True, stop=True)
        w_x = const.tile([128, 1024], dt, tag="wx")
        nc.scalar.activation(out=w_x[:], in_=pa[:], func=EXP)

        # ---- wy args: 8 matmuls [64,128]x[64,512] in rounds of 2 -> exp ----
        w_y = []
        for rnd in range(4):
            pa = psum_arg.tile([128, 1024], dt, tag="pa")
            for gg in range(2):
                grp = rnd * 2 + gg          # 4-chunk group 0..7
                ctile = coefT_y[grp // 2]
                r0 = 64 * (grp % 2)
                nc.tensor.matmul(pa[:, 512 * gg:512 * gg + 512],
                                 lhsT=ctile[r0:r0 + 64, :],
                                 rhs=basis_y2[grp % 2], start=True, stop=True)
            wt = const.tile([128, 1024], dtm, tag=f"wy{rnd}")
            nc.scalar.activation(out=wt[:], in_=pa[:], func=EXP)
            w_y.append(wt)

        # ---- main accumulation: out[py, (c,px)] += wy_j.T @ X_j ----
        # X built one chunk per op (3D APs — the 4D broadcast form ran at
        # ~2.7 cy/el on DVE); most chunks on DVE, every 4th on GpSimd.
        for j in range(NCHUNK):
            Xg = xpool.tile([128, 128], dtm, tag="X")
            Xg_r = Xg[:].rearrange("p (c x) -> p c x", c=4)
            ca_in = ca[:].rearrange("p (c j) -> p c j", c=4)[:, :, j]
            ca_in = ca_in.unsqueeze(2).broadcast_to([128, 4, 32])
            wx_in = w_x[:, 32 * j:32 * j + 32].unsqueeze(1)
            wx_in = wx_in.broadcast_to([128, 4, 32])
            eng = nc.gpsimd if j % 4 == 3 else nc.vector
            eng.tensor_mul(out=Xg_r, in0=ca_in, in1=wx_in)
            wyt = w_y[j // 8]
            oy = (j % 8) * 128
            nc.tensor.matmul(po[:], lhsT=wyt[:, oy:oy + 128], rhs=Xg[:],
                             start=(j == 0), stop=(j == NCHUNK - 1))

        # ---- normalize: img = num * (1/(den + 1e-8)) ----
        dent = work.tile([128, 32], dt, tag="dent")
        nc.vector.tensor_scalar(out=dent[:], in0=po[:, 96:128], scalar1=1e-8,
                                scalar2=None, op0=add)
        dr = work.tile([128, 32], dt, tag="dr")
        nc.vector.reciprocal(out=dr[:], in_=dent[:])
        img = work.tile([128, 96], dt, tag="img")
        img_r = img[:].rearrange("p (c x) -> p c x", c=3)
        num_r = po[:, 0:96].rearrange("p (c x) -> p c x", c=3)
        dr_b = dr[:].unsqueeze(1).broadcast_to([128, 3, 32])
        nc.vector.tensor_mul(out=img_r, in0=num_r, in1=dr_b)
        nc.sync.dma_start(out=out_d, in_=img[:])

    nc.compile()   # legalizes sync waits (HW allows 1/instruction) etc.
    return nc


def _host_prep(positions, colors, opacities, scales, qvec, tvec):
    """Build the 8 per-core input maps."""
    import ml_dtypes
    bf = ml_dtypes.bfloat16

    positions = np.ascontiguousarray(np.asarray(positions, dtype=F32))
    colors = np.ascontiguousarray(np.asarray(colors, dtype=F32))
    opacities = np.ascontiguousarray(np.asarray(opacities, dtype=F32))
    scales = np.ascontiguousarray(np.asarray(scales, dtype=F32))
    qvec = np.asarray(qvec, dtype=F32)
    tvec = np.asarray(tvec, dtype=F32)

    pos_v = positions.reshape(NCHUNK, 128, 3)
    sc_v = scales.reshape(NCHUNK, 128)
    pos4 = np.concatenate([pos_v[:, :, 0].T, pos_v[:, :, 1].T, pos_v[:, :, 2].T,
                           sc_v.T], axis=1).astype(F32)          # [128, 128]
    col_v = colors.reshape(NCHUNK, 128, 3)
    colors1 = np.concatenate([col_v[:, :, 0].T, col_v[:, :, 1].T, col_v[:, :, 2].T,
                              np.ones((128, NCHUNK), F32)], axis=1).astype(F32)
    opac = opacities.reshape(NCHUNK, 128).T.astype(F32)          # [128, 32]

    # folded pose matrices (centering: cx-64 = cy-64 = 0)
    rbs = []
    for p in range(NPOSE):
        R = _quat2mat(qvec[p])
        t = tvec[p].astype(np.float64)
        A = np.zeros((3, 4))
        A[0, :3] = FX * R[0] + (CX - 64.0) * R[2]
        A[0, 3] = FX * t[0] + (CX - 64.0) * t[2]
        A[1, :3] = FY * R[1] + (CY - 64.0) * R[2]
        A[1, 3] = FY * t[1] + (CY - 64.0) * t[2]
        A[2, :3] = R[2]
        A[2, 3] = t[2]
        rbs.append(A.reshape(1, 12).astype(F32))

    def basis_rows(q):
        """[16, len(q)] bf16 rows: p2h,p2l,p2h,p2l,p2h,p2l,q,q,q,1,1,1,0*4."""
        q = q.astype(F32)
        p2 = (q * q).astype(F32)
        p2h = p2.astype(bf)
        p2l = (p2 - p2h.astype(F32)).astype(F32).astype(bf)
        qb = q.astype(bf)
        one = np.ones_like(q, dtype=bf)
        zero = np.zeros_like(q, dtype=bf)
        return np.stack([p2h, p2l, p2h, p2l, p2h, p2l,
                         qb, qb, qb, one, one, one, zero, zero, zero, zero])

    # basis_y [64, 512]: 4-chunk groups; block-diag: rows 16*jin+r active in
    # cols 128*jin + py
    py = np.arange(128) - 64.0
    by_rows = basis_rows(py)                      # [16, 128]
    basis_y = np.zeros((64, 512), bf)
    for jin in range(4):
        basis_y[16 * jin:16 * jin + 16, 128 * jin:128 * jin + 128] = by_rows

    # basis_x per block b: [128, 256]: 8-chunk groups; cols 32*jin + px
    basis_xs = []
    for b in range(NBLK):
        px = np.arange(PXB * b, PXB * b + PXB) - 64.0
        bx_rows = basis_rows(px)                  # [16, 32]
        bx = np.zeros((128, 256), bf)
        for jin in range(8):
            bx[16 * jin:16 * jin + 16, 32 * jin:32 * jin + 32] = bx_rows
        basis_xs.append(bx)

    ident = np.eye(128, dtype=F32)

    in_maps = []
    for core in range(8):
        p, b = core // NBLK, core % NBLK
        inp128 = np.concatenate(
            [pos4, colors1, opac, ident, np.tile(rbs[p], (128, 1))],
            axis=1).astype(F32)                       # [128, 428]
        bas = np.zeros((128, 896), bf)
        bas[0:64, 0:512] = basis_y
        bas[64:128, 0:512] = basis_y
        bas[:, 512:768] = basis_xs[b]
        bas[:, 768:896] = np.eye(128, dtype=bf)
        in_maps.append({"inp128": inp128, "bas": bas})
    return in_maps


def _assemble(slabs):
    """slabs: list of 8 [128, 96] arrays -> [NPOSE*16, 3, 32, 32] output."""
    out = []
    for p in range(NPOSE):
        img = np.zeros((H, W, 3), F32)
        for b in range(NBLK):
            slab = slabs[p * NBLK + b]
            for c in range(3):
                img[:, PXB * b:PXB * b + PXB, c] = slab[:, 32 * c:32 * c + 32]
        tiles = img.reshape(H * W, 3).reshape(16, 1024, 3)
        tiles = tiles.transpose(0, 2, 1).reshape(16, 3, 32, 32)
        out.append(tiles)
    return np.concatenate(out, axis=0).astype(F32)


def kernel(positions, colors, opacities, scales, qvec, tvec, _trace=False):
    from concourse.bass_utils import run_bass_kernel_spmd

    if "nc" not in _CACHE:
        _CACHE["nc"] = _build_program()
    nc = _CACHE["nc"]

    in_maps = _host_prep(positions, colors, opacities, scales, qvec, tvec)
    res = run_bass_kernel_spmd(nc, in_maps, core_ids=list(range(8)),
                               trace=_trace)
    slabs = [np.asarray(res.results[c]["out"]) for c in range(8)]
    out = _assemble(slabs)
    if _trace:
        _CACHE["last_result"] = res
    return out



# revision 2
# speedup vs baseline: 1.4766x; 1.4766x over previous
"""Trainium2 Bass kernel for the differentiable gaussian-splat renderer.

Full-input contract: kernel(**inputs) takes the unsharded inputs and returns
the full [2*16, 3, 32, 32] output.

Math (per pose):
    cam = positions @ R.T + t ;  pj = (fx*cam_x/cam_z + cx, fy*cam_y/cam_z + cy)
    w[n, p] = op_n * exp(-0.5*((px-ax_n)^2 + (py-ay_n)^2)/s_n^2)
    img = (w.T @ colors) / (w.T @ 1 + 1e-8)

The gaussian weight is separable: w = wx[n,px] * wy[n,py] (opacity folded into
wx via ln(op) added to the constant coefficient), so instead of N*HW
exponentials we need N*(W + H) and the pixel accumulation becomes a
K=128-chunked matmul  out[py, (c,px)] += wy_chunk.T @ (car_chunk (*) wx_chunk).

Sharding: 8 independent cores = 2 poses x 4 px-column blocks (32 px each).
No collectives; each core computes all 4096 gaussians for its (pose, px-block)
and writes a [128, 96] slab = (py, 32c+px_local). Host reassembles.

All O(N) per-gaussian geometry (projection, quadratic coefficients
[g, -2*g*ax', g*ax'^2 + ln(op)], their exact 3-way bf16 splits, and the
transposed coefficient packs the PE consumes) is precomputed on host; the
device does the O(N*(W+H)) separable evaluation (arg matmuls + exp) and the
O(N*H*W/8) accumulation. The color array is pre-expanded on host to
car[p, (j,c,px)] so the X-build is a contiguous bf16 multiply (DVE fast
path) instead of a slow stride-0-inner broadcast.
"""

import numpy as np

H = 128
W = 128
FX = 120.0
FY = 120.0
CX = 64.0
CY = 64.0
N = 4096
NCHUNK = 32          # 4096 / 128
NPOSE = 2
PXB = 32             # px columns per core
NBLK = 4             # px blocks
NG = 4               # 8-chunk groups
F32 = np.float32

_CACHE = {}


def _quat2mat(q):
    q = np.asarray(q, dtype=np.float64)
    q = q / np.linalg.norm(q)
    w, x, y, z = q
    return np.array([
        [1 - 2 * (y * y + z * z), 2 * (x * y - z * w), 2 * (x * z + y * w)],
        [2 * (x * y + z * w), 1 - 2 * (x * x + z * z), 2 * (y * z - x * w)],
        [2 * (x * z - y * w), 2 * (y * z + x * w), 1 - 2 * (x * x + y * y)],
    ])


def _build_program():
    """Build the SPMD Bass/Tile program (same program on every core)."""
    import concourse.bacc as bacc
    import concourse.tile as tile
    import concourse.mybir as mybir
    from contextlib import ExitStack

    dt = mybir.dt.float32
    bf = mybir.dt.bfloat16
    nc = bacc.Bacc()

    # ---- DRAM I/O (per-core shapes) ----
    # basA (bf16): cols 0:512 coefT_x (4 tiles of [128,128]) | 512:768 basis_x
    basA_d = nc.dram_tensor("basA", [128, 768], bf, kind="ExternalInput").ap()
    # basB (bf16): cols 0:512 coefT_y (4 tiles) | 512:1536 basis_y8
    basB_d = nc.dram_tensor("basB", [128, 1536], bf, kind="ExternalInput").ap()
    # car (bf16): col 1024*g + 128*jrel + 32*c + px = color_c(gauss) (c=3: 1.0)
    car_d = [nc.dram_tensor(f"car{g}", [128, 1024], bf, kind="ExternalInput").ap()
             for g in range(NG)]
    out_d = nc.dram_tensor("out", [128, 96], dt, kind="ExternalOutput").ap()

    add = mybir.AluOpType.add
    EXP = mybir.ActivationFunctionType.Exp

    with tile.TileContext(nc) as tc, ExitStack() as ctx:
        const = ctx.enter_context(tc.tile_pool(name="const", bufs=1))
        work = ctx.enter_context(tc.tile_pool(name="work", bufs=1))
        psum_ax = ctx.enter_context(tc.tile_pool(name="psum_ax", bufs=1, space="PSUM"))
        psum_ay = ctx.enter_context(tc.tile_pool(name="psum_ay", bufs=2, space="PSUM"))
        psum_out = ctx.enter_context(tc.tile_pool(name="psum_out", bufs=1, space="PSUM"))

        po = psum_out.tile([128, 128], dt, tag="po")  # claim psum bank 0 first

        basA = const.tile([128, 768], bf, tag="basA")
        nc.sync.dma_start(out=basA[:], in_=basA_d)
        car = const.tile([128, 4096], bf, tag="car")
        nc.sync.dma_start(out=car[:, 0:1024], in_=car_d[0])
        basB = const.tile([128, 1536], bf, tag="basB")
        nc.sync.dma_start(out=basB[:], in_=basB_d)
        for g in range(1, NG):
            nc.sync.dma_start(out=car[:, 1024 * g:1024 * g + 1024], in_=car_d[g])

        basis_x = basA[:, 512:768]

        # ---- wx args: 4 matmuls [128,128]x[128,256] -> one exp -> bf16 ----
        pa_x = psum_ax.tile([128, 1024], dt, tag="pax")
        for t in range(4):
            nc.tensor.matmul(pa_x[:, 256 * t:256 * t + 256],
                             lhsT=basA[:, 128 * t:128 * t + 128],
                             rhs=basis_x, start=True, stop=True)
        w_x = const.tile([128, 1024], bf, tag="wx")
        nc.scalar.activation(out=w_x[:], in_=pa_x[:], func=EXP)

        # ---- X build: per 8-chunk group, X = car (*) wx  (all bf16,
        #      inner-dim contiguous -> DVE fast path) ----
        xs = []
        for g in range(NG):
            Xg = const.tile([128, 1024], bf, tag=f"X{g}")
            car_v = car[:, 1024 * g:1024 * g + 1024].rearrange(
                "p (j c x) -> p j c x", j=8, c=4)
            wx_v = w_x[:, 256 * g:256 * g + 256].rearrange(
                "p (j x) -> p j x", j=8).unsqueeze(2).broadcast_to([128, 8, 4, 32])
            out_v = Xg[:].rearrange("p (j c x) -> p j c x", j=8, c=4)
            nc.vector.tensor_mul(out=out_v, in0=car_v, in1=wx_v)
            xs.append(Xg)

        # ---- wy args (2 matmuls per group into ping-pong psum) + exp,
        #      interleaved with the accumulation batches ----
        def wy_args(g):
            pa = psum_ay.tile([128, 1024], dt, tag="pay")
            for h in range(2):
                nc.tensor.matmul(pa[:, 512 * h:512 * h + 512],
                                 lhsT=basB[:, 128 * g:128 * g + 128],
                                 rhs=basB[:, 512 + 512 * h:1024 + 512 * h],
                                 start=True, stop=True)
            return pa

        def wy_exp(pa):
            wt = const.tile([128, 1024], bf, tag="wy")
            nc.scalar.activation(out=wt[:], in_=pa[:], func=EXP)
            return wt

        pa0 = wy_args(0)
        wy0 = wy_exp(pa0)
        pa1 = wy_args(1)
        wys = [wy0, wy_exp(pa1)]

        for g in range(NG):
            wyt = wys[g]
            for jrel in range(8):
                j = 8 * g + jrel
                nc.tensor.matmul(po[:],
                                 lhsT=wyt[:, 128 * jrel:128 * jrel + 128],
                                 rhs=xs[g][:, 128 * jrel:128 * jrel + 128],
                                 start=(j == 0), stop=(j == NCHUNK - 1))
            if g + 2 < NG:
                pa = wy_args(g + 2)
                wys.append(wy_exp(pa))

        # ---- normalize: img = num * (1/(den + 1e-8)) ----
        dent = work.tile([128, 32], dt, tag="dent")
        nc.vector.tensor_scalar(out=dent[:], in0=po[:, 96:128], scalar1=1e-8,
                                scalar2=None, op0=add)
        dr = work.tile([128, 32], dt, tag="dr")
        nc.vector.reciprocal(out=dr[:], in_=dent[:])
        img = work.tile([128, 96], dt, tag="img")
        img_r = img[:].rearrange("p (c x) -> p c x", c=3)
        num_r = po[:, 0:96].rearrange("p (c x) -> p c x", c=3)
        dr_b = dr[:].unsqueeze(1).broadcast_to([128, 3, 32])
        nc.vector.tensor_mul(out=img_r, in0=num_r, in1=dr_b)
        nc.sync.dma_start(out=out_d, in_=img[:])

    nc.compile()   # legalizes sync waits (HW allows 1/instruction) etc.
    return nc


def _split3(v, bf):
    """Exact-ish 3-way bf16 split of float64/float32 array v."""
    v = v.astype(F32)
    p1 = v.astype(bf)
    r1 = (v - p1.astype(F32)).astype(F32)
    p2 = r1.astype(bf)
    r2 = (r1 - p2.astype(F32)).astype(F32)
    p3 = r2.astype(bf)
    return p1, p2, p3


def _host_prep(positions, colors, opacities, scales, qvec, tvec):
    """Build the 8 per-core input maps (all O(N) numpy work)."""
    import ml_dtypes
    bf = ml_dtypes.bfloat16

    positions = np.asarray(positions, dtype=np.float64)
    colors = np.asarray(colors, dtype=F32)
    opacities = np.asarray(opacities, dtype=np.float64)
    scales = np.asarray(scales, dtype=np.float64)
    qvec = np.asarray(qvec, dtype=F32)
    tvec = np.asarray(tvec, dtype=F32)

    g_coef = -0.5 / (scales[:, 0] ** 2)                       # [N]
    lnop = np.log(np.maximum(opacities[:, 0], 1e-300))        # [N]

    def basis_rows(q):
        """[16, len(q)] bf16 rows: p2h,p2l,p2h,p2l,p2h,p2l,q,q,q,1,1,1,0*4."""
        q = q.astype(F32)
        p2 = (q * q).astype(F32)
        p2h = p2.astype(bf)
        p2l = (p2 - p2h.astype(F32)).astype(F32).astype(bf)
        qb = q.astype(bf)
        one = np.ones_like(q, dtype=bf)
        zero = np.zeros_like(q, dtype=bf)
        return np.stack([p2h, p2l, p2h, p2l, p2h, p2l,
                         qb, qb, qb, one, one, one, zero, zero, zero, zero])

    def coefT(A, B, C):
        """[N] coefs -> [128, 512] bf16 transposed pack.

        Tile t (cols 128t:128t+128): row 16*jrel + r, col = n_in_chunk,
        rows r: (A1,A1,A2,A2,A3,A3,B1,B2,B3,C1,C2,C3,0,0,0,0)."""
        a1, a2, a3 = _split3(A, bf)
        b1, b2, b3 = _split3(B, bf)
        c1, c2, c3 = _split3(C, bf)
        zero = np.zeros_like(a1)
        rows = np.stack([a1, a1, a2, a2, a3, a3, b1, b2, b3, c1, c2, c3,
                         zero, zero, zero, zero])            # [16, N]
        # [16, 32 chunks, 128 n] -> per tile t: [16*8jrel rows, 128]
        rows = rows.reshape(16, NCHUNK, 128)
        pack = np.zeros((128, 512), bf)
        for j in range(NCHUNK):
            t, jrel = j // 8, j % 8
            pack[16 * jrel:16 * jrel + 16, 128 * t:128 * t + 128] = rows[:, j, :]
        return pack

    # basis_y8 [128, 1024]: block-diag, rows 16*jrel+r, cols 128*jrel + py
    py = np.arange(128) - CY
    by_rows = basis_rows(py)                      # [16, 128]
    basis_y8 = np.zeros((128, 1024), bf)
    for jrel in range(8):
        basis_y8[16 * jrel:16 * jrel + 16, 128 * jrel:128 * jrel + 128] = by_rows

    # car [128, 4096] bf16: col 128*j + 32*c + px = color_c(128j+p) (c=3: 1)
    colc = np.concatenate([colors, np.ones((N, 1), F32)], axis=1)  # [N, 4]
    car = colc.reshape(NCHUNK, 128, 4).transpose(1, 0, 2)          # [128, j, c]
    car = np.repeat(car[:, :, :, None], PXB, axis=3)               # [128,j,c,px]
    car = np.ascontiguousarray(car.reshape(128, NCHUNK * 128)).astype(bf)

    in_maps = []
    for p in range(NPOSE):
        R = _quat2mat(qvec[p])
        t64 = tvec[p].astype(np.float64)
        u = positions @ (FX * R[0]) + FX * t64[0]
        v = positions @ (FY * R[1]) + FY * t64[1]
        zc = positions @ R[2] + t64[2]
        ax = u / zc + CX          # absolute px coords of gaussian center
        ay = v / zc + CY
        ayc = ay - CY             # y centered at 64
        coefT_y = coefT(g_coef, -2.0 * g_coef * ayc, g_coef * ayc * ayc)
        basB = np.zeros((128, 1536), bf)
        basB[:, 0:512] = coefT_y
        basB[:, 512:1536] = basis_y8

        for b in range(NBLK):
            cb = 32.0 * b + 16.0                  # block center
            axc = ax - cb
            coefT_x = coefT(g_coef, -2.0 * g_coef * axc,
                            g_coef * axc * axc + lnop)
            px = np.arange(PXB * b, PXB * b + PXB) - cb   # in [-16, 16)
            bx_rows = basis_rows(px)                      # [16, 32]
            basis_x = np.zeros((128, 256), bf)
            for jrel in range(8):
                basis_x[16 * jrel:16 * jrel + 16,
                        32 * jrel:32 * jrel + 32] = bx_rows
            basA = np.zeros((128, 768), bf)
            basA[:, 0:512] = coefT_x
            basA[:, 512:768] = basis_x
            m = {"basA": basA, "basB": basB}
            for g in range(NG):
                m[f"car{g}"] = np.ascontiguousarray(
                    car[:, 1024 * g:1024 * g + 1024])
            in_maps.append(m)
    return in_maps


def _assemble(slabs):
    """slabs: list of 8 [128, 96] arrays -> [NPOSE*16, 3, 32, 32] output."""
    out = []
    for p in range(NPOSE):
        img = np.zeros((H, W, 3), F32)
        for b in range(NBLK):
            slab = slabs[p * NBLK + b]
            for c in range(3):
                img[:, PXB * b:PXB * b + PXB, c] = slab[:, 32 * c:32 * c + 32]
        tiles = img.reshape(H * W, 3).reshape(16, 1024, 3)
        tiles = tiles.transpose(0, 2, 1).reshape(16, 3, 32, 32)
        out.append(tiles)
    return np.concatenate(out, axis=0).astype(F32)


def kernel(positions, colors, opacities, scales, qvec, tvec, _trace=False):
    from concourse.bass_utils import run_bass_kernel_spmd

    if "nc" not in _CACHE:
        _CACHE["nc"] = _build_program()
    nc = _CACHE["nc"]

    in_maps = _host_prep(positions, colors, opacities, scales, qvec, tvec)
    res = run_bass_kernel_spmd(nc, in_maps, core_ids=list(range(8)),
                               trace=_trace)
    slabs = [np.asarray(res.results[c]["out"]) for c in range(8)]
    out = _assemble(slabs)
    if _trace:
        _CACHE["last_result"] = res
    return out


# revision 6
# speedup vs baseline: 1.7276x; 1.1700x over previous
"""Trainium2 Bass kernel for the differentiable gaussian-splat renderer.

Full-input contract: kernel(**inputs) takes the unsharded inputs and returns
the full [2*16, 3, 32, 32] output.

Math (per pose):
    cam = positions @ R.T + t ;  pj = (fx*cam_x/cam_z + cx, fy*cam_y/cam_z + cy)
    w[n, p] = op_n * exp(-0.5*((px-ax_n)^2 + (py-ay_n)^2)/s_n^2)
    img = (w.T @ colors) / (w.T @ 1 + 1e-8)

The gaussian weight is separable: w = wx[n,px] * wy[n,py] (opacity folded into
wx via ln(op) added to the constant coefficient), so instead of N*HW
exponentials we need N*(W + H) and the pixel accumulation becomes a
K=128-chunked matmul  out[py, (c,px)] += wy_chunk.T @ (car_chunk (*) wx_chunk).

Sharding: 8 independent cores = 2 poses x 4 px-column blocks (32 px each).
No collectives; each core computes all 4096 gaussians for its (pose, px-block)
and writes a [128, 96] slab = (py, 32c+px_local). Host reassembles.

All O(N) per-gaussian geometry (projection, quadratic coefficients
[g, -2*g*ax', g*ax'^2 + ln(op)], their exact 3-way bf16 splits, and the
transposed coefficient packs the PE consumes) is precomputed on host; the
device does the O(N*(W+H)) separable evaluation (arg matmuls + exp) and the
O(N*H*W/8) accumulation. The color array is pre-expanded on host to
car[p, (j,c,px)] so the X-build is a contiguous bf16 multiply (DVE fast
path) instead of a slow stride-0-inner broadcast.
"""

import numpy as np

H = 128
W = 128
FX = 120.0
FY = 120.0
CX = 64.0
CY = 64.0
N = 4096
NCHUNK = 32          # 4096 / 128
NPOSE = 2
PXB = 32             # px columns per core
NBLK = 4             # px blocks
NG = 4               # 8-chunk groups
F32 = np.float32

_CACHE = {}


def _quat2mat(q):
    q = np.asarray(q, dtype=np.float64)
    q = q / np.linalg.norm(q)
    w, x, y, z = q
    return np.array([
        [1 - 2 * (y * y + z * z), 2 * (x * y - z * w), 2 * (x * z + y * w)],
        [2 * (x * y + z * w), 1 - 2 * (x * x + z * z), 2 * (y * z - x * w)],
        [2 * (x * z - y * w), 2 * (y * z + x * w), 1 - 2 * (x * x + y * y)],
    ])


def _build_program():
    """Build the SPMD Bass/Tile program (same program on every core)."""
    import concourse.bacc as bacc
    import concourse.tile as tile
    import concourse.mybir as mybir
    from contextlib import ExitStack

    dt = mybir.dt.float32
    bf = mybir.dt.bfloat16
    nc = bacc.Bacc()

    # ---- DRAM I/O (per-core shapes) ----
    # basA (bf16): cols 0:512 coefT_x (4 tiles of [128,128]) | 512:768 basis_x
    basA_d = nc.dram_tensor("basA", [128, 768], bf, kind="ExternalInput").ap()
    # basB (bf16): cols 0:512 coefT_y (4 tiles) | 512:1536 basis_y8
    basB_d = nc.dram_tensor("basB", [128, 1536], bf, kind="ExternalInput").ap()
    # car (bf16): col 1024*g + 128*jrel + 32*c + px = color_c(gauss) (c=3: 1.0)
    car_d = [nc.dram_tensor(f"car{g}", [128, 1024], bf, kind="ExternalInput").ap()
             for g in range(NG)]
    out_d = nc.dram_tensor("out", [128, 96], dt, kind="ExternalOutput").ap()

    add = mybir.AluOpType.add
    EXP = mybir.ActivationFunctionType.Exp

    with tile.TileContext(nc) as tc, ExitStack() as ctx:
        const = ctx.enter_context(tc.tile_pool(name="const", bufs=1))
        psum_arg = ctx.enter_context(tc.tile_pool(name="psum_arg", bufs=3, space="PSUM"))
        psum_out = ctx.enter_context(tc.tile_pool(name="psum_out", bufs=1, space="PSUM"))

        po = psum_out.tile([128, 128], dt, tag="po")  # claim psum bank 0 first

        basA = const.tile([128, 768], bf, tag="basA")
        nc.sync.dma_start(out=basA[:], in_=basA_d)
        basB = const.tile([128, 1536], bf, tag="basB")
        nc.sync.dma_start(out=basB[:], in_=basB_d)
        car = const.tile([128, 4096], bf, tag="car")
        for g in range(NG):
            nc.sync.dma_start(out=car[:, 1024 * g:1024 * g + 1024], in_=car_d[g])

        basis_x = basA[:, 512:768]

        # ---- wx args: 4 matmuls [128,128]x[128,256] -> one exp -> bf16 ----
        pa_x = psum_arg.tile([128, 1024], dt, tag="pa")
        for t in range(4):
            nc.tensor.matmul(pa_x[:, 256 * t:256 * t + 256],
                             lhsT=basA[:, 128 * t:128 * t + 128],
                             rhs=basis_x, start=True, stop=True)
        w_x = const.tile([128, 1024], bf, tag="wx")
        nc.scalar.activation(out=w_x[:], in_=pa_x[:], func=EXP)

        # ---- X build: per 8-chunk group, X = car (*) wx  (all bf16,
        #      inner-dim contiguous -> DVE fast path) ----
        xs = []
        for g in range(NG):
            Xg = const.tile([128, 1024], bf, tag=f"X{g}")
            car_v = car[:, 1024 * g:1024 * g + 1024].rearrange(
                "p (j c x) -> p j c x", j=8, c=4)
            wx_v = w_x[:, 256 * g:256 * g + 256].rearrange(
                "p (j x) -> p j x", j=8).unsqueeze(2).broadcast_to([128, 8, 4, 32])
            out_v = Xg[:].rearrange("p (j c x) -> p j c x", j=8, c=4)
            nc.vector.tensor_mul(out=out_v, in0=car_v, in1=wx_v)
            xs.append(Xg)

        # ---- wy args (2 matmuls per group) + exp: all emitted upfront so
        #      no acc batch sits between a wy matmul and its exp (the sem
        #      update otherwise lands late and stalls the scalar engine) ----
        wys = []
        for g in range(NG):
            pa = psum_arg.tile([128, 1024], dt, tag="pa")
            for h in range(2):
                nc.tensor.matmul(pa[:, 512 * h:512 * h + 512],
                                 lhsT=basB[:, 128 * g:128 * g + 128],
                                 rhs=basB[:, 512 + 512 * h:1024 + 512 * h],
                                 start=True, stop=True)
            wt = const.tile([128, 1024], bf, tag=f"wy{g}")
            nc.scalar.activation(out=wt[:], in_=pa[:], func=EXP)
            wys.append(wt)

        for g in range(NG):
            wyt = wys[g]
            for jrel in range(8):
                j = 8 * g + jrel
                nc.tensor.matmul(po[:],
                                 lhsT=wyt[:, 128 * jrel:128 * jrel + 128],
                                 rhs=xs[g][:, 128 * jrel:128 * jrel + 128],
                                 start=(j == 0), stop=(j == NCHUNK - 1))

        # ---- normalize: img = num * (1/(den + 1e-8)) ----
        dent = const.tile([128, 32], dt, tag="dent")
        nc.vector.tensor_scalar(out=dent[:], in0=po[:, 96:128], scalar1=1e-8,
                                scalar2=None, op0=add)
        dr = const.tile([128, 32], dt, tag="dr")
        nc.vector.reciprocal(out=dr[:], in_=dent[:])
        img = const.tile([128, 96], dt, tag="img")
        img_r = img[:].rearrange("p (c x) -> p c x", c=3)
        num_r = po[:, 0:96].rearrange("p (c x) -> p c x", c=3)
        dr_b = dr[:].unsqueeze(1).broadcast_to([128, 3, 32])
        nc.vector.tensor_mul(out=img_r, in0=num_r, in1=dr_b)
        nc.sync.dma_start(out=out_d, in_=img[:])

    nc.compile()   # legalizes sync waits (HW allows 1/instruction) etc.
    return nc


def _split3(v, bf):
    """Exact-ish 3-way bf16 split of float64/float32 array v."""
    v = v.astype(F32)
    p1 = v.astype(bf)
    r1 = (v - p1.astype(F32)).astype(F32)
    p2 = r1.astype(bf)
    r2 = (r1 - p2.astype(F32)).astype(F32)
    p3 = r2.astype(bf)
    return p1, p2, p3


def _host_prep(positions, colors, opacities, scales, qvec, tvec):
    """Build the 8 per-core input maps (all O(N) numpy work)."""
    import ml_dtypes
    bf = ml_dtypes.bfloat16

    positions = np.asarray(positions, dtype=np.float64)
    colors = np.asarray(colors, dtype=F32)
    opacities = np.asarray(opacities, dtype=np.float64)
    scales = np.asarray(scales, dtype=np.float64)
    qvec = np.asarray(qvec, dtype=F32)
    tvec = np.asarray(tvec, dtype=F32)

    g_coef = -0.5 / (scales[:, 0] ** 2)                       # [N]
    lnop = np.log(np.maximum(opacities[:, 0], 1e-300))        # [N]

    def basis_rows(q):
        """[16, len(q)] bf16 rows: p2h,p2l,p2h,p2l,p2h,p2l,q,q,q,1,1,1,0*4."""
        q = q.astype(F32)
        p2 = (q * q).astype(F32)
        p2h = p2.astype(bf)
        p2l = (p2 - p2h.astype(F32)).astype(F32).astype(bf)
        qb = q.astype(bf)
        one = np.ones_like(q, dtype=bf)
        zero = np.zeros_like(q, dtype=bf)
        return np.stack([p2h, p2l, p2h, p2l, p2h, p2l,
                         qb, qb, qb, one, one, one, zero, zero, zero, zero])

    def coefT(A, B, C):
        """[N] coefs -> [128, 512] bf16 transposed pack.

        Tile t (cols 128t:128t+128): row 16*jrel + r, col = n_in_chunk,
        rows r: (A1,A1,A2,A2,A3,A3,B1,B2,B3,C1,C2,C3,0,0,0,0)."""
        a1, a2, a3 = _split3(A, bf)
        b1, b2, b3 = _split3(B, bf)
        c1, c2, c3 = _split3(C, bf)
        zero = np.zeros_like(a1)
        rows = np.stack([a1, a1, a2, a2, a3, a3, b1, b2, b3, c1, c2, c3,
                         zero, zero, zero, zero])            # [16, N]
        # [16, 32 chunks, 128 n] -> per tile t: [16*8jrel rows, 128]
        rows = rows.reshape(16, NCHUNK, 128)
        pack = np.zeros((128, 512), bf)
        for j in range(NCHUNK):
            t, jrel = j // 8, j % 8
            pack[16 * jrel:16 * jrel + 16, 128 * t:128 * t + 128] = rows[:, j, :]
        return pack

    # basis_y8 [128, 1024]: block-diag, rows 16*jrel+r, cols 128*jrel + py
    py = np.arange(128) - CY
    by_rows = basis_rows(py)                      # [16, 128]
    basis_y8 = np.zeros((128, 1024), bf)
    for jrel in range(8):
        basis_y8[16 * jrel:16 * jrel + 16, 128 * jrel:128 * jrel + 128] = by_rows

    # car [128, 4096] bf16: col 128*j + 32*c + px = color_c(128j+p) (c=3: 1)
    colc = np.concatenate([colors, np.ones((N, 1), F32)], axis=1)  # [N, 4]
    car = colc.reshape(NCHUNK, 128, 4).transpose(1, 0, 2)          # [128, j, c]
    car = np.repeat(car[:, :, :, None], PXB, axis=3)               # [128,j,c,px]
    car = np.ascontiguousarray(car.reshape(128, NCHUNK * 128)).astype(bf)

    in_maps = []
    for p in range(NPOSE):
        R = _quat2mat(qvec[p])
        t64 = tvec[p].astype(np.float64)
        u = positions @ (FX * R[0]) + FX * t64[0]
        v = positions @ (FY * R[1]) + FY * t64[1]
        zc = positions @ R[2] + t64[2]
        ax = u / zc + CX          # absolute px coords of gaussian center
        ay = v / zc + CY
        ayc = ay - CY             # y centered at 64
        coefT_y = coefT(g_coef, -2.0 * g_coef * ayc, g_coef * ayc * ayc)
        basB = np.zeros((128, 1536), bf)
        basB[:, 0:512] = coefT_y
        basB[:, 512:1536] = basis_y8

        for b in range(NBLK):
            cb = 32.0 * b + 16.0                  # block center
            axc = ax - cb
            coefT_x = coefT(g_coef, -2.0 * g_coef * axc,
                            g_coef * axc * axc + lnop)
            px = np.arange(PXB * b, PXB * b + PXB) - cb   # in [-16, 16)
            bx_rows = basis_rows(px)                      # [16, 32]
            basis_x = np.zeros((128, 256), bf)
            for jrel in range(8):
                basis_x[16 * jrel:16 * jrel + 16,
                        32 * jrel:32 * jrel + 32] = bx_rows
            basA = np.zeros((128, 768), bf)
            basA[:, 0:512] = coefT_x
            basA[:, 512:768] = basis_x
            m = {"basA": basA, "basB": basB}
            for g in range(NG):
                m[f"car{g}"] = np.ascontiguousarray(
                    car[:, 1024 * g:1024 * g + 1024])
            in_maps.append(m)
    return in_maps


def _assemble(slabs):
    """slabs: list of 8 [128, 96] arrays -> [NPOSE*16, 3, 32, 32] output."""
    out = []
    for p in range(NPOSE):
        img = np.zeros((H, W, 3), F32)
        for b in range(NBLK):
            slab = slabs[p * NBLK + b]
            for c in range(3):
                img[:, PXB * b:PXB * b + PXB, c] = slab[:, 32 * c:32 * c + 32]
        tiles = img.reshape(H * W, 3).reshape(16, 1024, 3)
        tiles = tiles.transpose(0, 2, 1).reshape(16, 3, 32, 32)
        out.append(tiles)
    return np.concatenate(out, axis=0).astype(F32)


def kernel(positions, colors, opacities, scales, qvec, tvec, _trace=False):
    from concourse.bass_utils import run_bass_kernel_spmd

    if "nc" not in _CACHE:
        _CACHE["nc"] = _build_program()
    nc = _CACHE["nc"]

    in_maps = _host_prep(positions, colors, opacities, scales, qvec, tvec)
    res = run_bass_kernel_spmd(nc, in_maps, core_ids=list(range(8)),
                               trace=_trace)
    slabs = [np.asarray(res.results[c]["out"]) for c in range(8)]
    out = _assemble(slabs)
    if _trace:
        _CACHE["last_result"] = res
    return out
